# revision 1
# baseline (speedup 1.0000x reference)
"""Trainium2 Bass kernel for nn_DecoderLayer (conv-QKV attention + conv FFN).

Sharding: 8 cores = 4 batches x 2 token-halves. Each core computes the full
attention + FFN for 1024 tokens of one batch element. The 4 halo context
tokens each core's FFN conv needs (s-2, s-1, e+1, e+2) are computed on the
host (~0.4% of total FLOPs) and passed in.

v2 design (ACT-bound):
  All matmul operands are bf16 (fp32 streams at 2 cycles/row on the PE;
  bf16 at 1). The softmax exp on the Scalar engine (16.8M elements/core,
  ~1.15us per [128,1024] tile) is the kernel floor, so the attention loop
  is built as a back-to-back ACT chain: double-buffered score PSUM
  (2-way row-tiled score pairs), software-pipelined ctx matmuls (col-tiled
  2-way concurrent, M=33 with the softmax-denominator ones row), and ALL
  conv work (QKV + FFN conv1/conv2) emitted as budget-metered "filler"
  matmuls inside the attention loop so the PE uses the exp-wait windows.

Layouts:
  x, q, k       channel-major  [ch(part), tok(free)]  bf16
  v             token-major, ones-augmented per head ([kt, 33] lhsT slices
                -> one M=33 matmul accumulates ctx rows 0..31 + softmax
                denominator in row 32)
  scores        [kt(part), q(free)] pairs of heads in one [128,1024] PSUM
                tile so exp is a single wide ACT op -> probs bf16 SBUF
  ctx accum     one PSUM bank per head pair: head A at partitions 0..32,
                head B at partitions 64..96 (col-tiled concurrent matmuls)
  y1            channel-major bf16; y2/LN/residual token-major f32
"""

import contextlib

import ml_dtypes
import numpy as np

import concourse.bass as bass
import concourse.mybir as mybir
import concourse.tile as tile
from concourse import bacc, bass_utils

F32 = mybir.dt.float32
BF16 = mybir.dt.bfloat16
AF = mybir.ActivationFunctionType
ALU = mybir.AluOpType
NPBF16 = ml_dtypes.bfloat16

B, L, D = 4, 2048, 256
H, DH, DFF = 8, 32, 1024
LN_EPS = 1e-5
HALF = L // 2          # tokens per core
NCORES = 8
INV_SQRT_H = 1.0 / np.sqrt(np.float32(H))

# Priority offset for the attention chain (scores/exp/ctx/normalize): makes
# the Tile scheduler treat it as issued far earlier than the conv "fillers".
PRIO_OFF = 10 ** 6

_cache = {}


def _bcast_ap(t, row, width, parts):
    """DRAM row -> all-partition broadcast AP."""
    a = t[row : row + 1, :width]
    return bass.AP(tensor=a.tensor, offset=a.offset, ap=[[0, parts]] + a.ap[1:])


def build_nc():
    nc = bacc.Bacc("TRN2", target_bir_lowering=False, debug=False)

    # ---- DRAM I/O (per-core) ----
    xcm = nc.dram_tensor("xcm", [D, L + 4], BF16, kind="ExternalInput")
    xres = nc.dram_tensor("xres", [HALF, D], F32, kind="ExternalInput")
    wq = nc.dram_tensor("wq", [3, D, D], BF16, kind="ExternalInput")
    wk = nc.dram_tensor("wk", [3, D, D], BF16, kind="ExternalInput")
    wv = nc.dram_tensor("wv", [3, D, D], BF16, kind="ExternalInput")
    w1 = nc.dram_tensor("w1", [3, D, DFF], BF16, kind="ExternalInput")
    w2 = nc.dram_tensor("w2", [3, DFF, D], BF16, kind="ExternalInput")
    bqv = nc.dram_tensor("bqv", [3, D], F32, kind="ExternalInput")   # bq, bk, bv
    b1d = nc.dram_tensor("b1d", [DFF], F32, kind="ExternalInput")
    b2d = nc.dram_tensor("b2d", [D], F32, kind="ExternalInput")
    gam = nc.dram_tensor("gam", [D], F32, kind="ExternalInput")
    ctxh = nc.dram_tensor("ctxh", [D, 4], BF16, kind="ExternalInput")
    out = nc.dram_tensor("out", [HALF, D], F32, kind="ExternalOutput")

    b1r = b1d.ap().rearrange("(a b) -> a b", b=1)    # [1024, 1]

    with tile.TileContext(nc) as tc:
        est = contextlib.ExitStack()
        with est:
            # ================= persistent SBUF =================
            pw = est.enter_context(tc.tile_pool(name="pw", bufs=1))
            pa = est.enter_context(tc.tile_pool(name="pa_acts", bufs=1))

            # x: split columns so the first conv chunks start before the
            # whole row block lands
            x_sb = []
            for it in range(2):
                t = pw.tile([128, L + 4], BF16, name=f"x{it}", tag=f"x{it}")
                nc.sync.dma_start(
                    t[:, 0:700], xcm.ap()[128 * it : 128 * it + 128, 0:700]
                )
                x_sb.append(t)
            for c0, c1 in ((700, 1400), (1400, L + 4)):
                for it in range(2):
                    nc.sync.dma_start(
                        x_sb[it][:, c0:c1],
                        xcm.ap()[128 * it : 128 * it + 128, c0:c1],
                    )
            # consolidated weight loads: one DMA per (tensor, it) with a
            # permuted 3D DRAM AP [i(part), k, o]; spread across engine
            # queues so the descriptor rings run in parallel.
            def wload(queue, dram, it, kdim, odim):
                src = dram.ap()
                ap3 = bass.AP(
                    tensor=src.tensor,
                    offset=src.offset + 128 * it * odim,
                    ap=[[odim, 128], [kdim * odim, 3], [1, odim]],
                )
                t = pw.tile([128, 3 * odim], BF16,
                            name=f"{dram.name}{it}", tag=f"{dram.name}{it}")
                queue.dma_start(t[:], ap3)
                return t

            wq_sb = [wload(nc.scalar, wq, it, D, D) for it in range(2)]
            wk_sb = [wload(nc.scalar, wk, it, D, D) for it in range(2)]
            wv_sb = [wload(nc.scalar, wv, it, D, D) for it in range(2)]
            w1_sb = [wload(nc.sync, w1, it, D, DFF) for it in range(2)]
            w2_sb = [wload(nc.sync, w2, it, DFF, D) for it in range(8)]

            def WQi(k, it, c0=0, w=D):
                return wq_sb[it][:, D * k + c0 : D * k + c0 + w]

            def WKi(k, it, c0=0, w=D):
                return wk_sb[it][:, D * k + c0 : D * k + c0 + w]

            def WVi(k, it, c0=0, w=D):
                return wv_sb[it][:, D * k + c0 : D * k + c0 + w]

            def W1i(k, it, c0=0, w=DFF):
                return w1_sb[it][:, DFF * k + c0 : DFF * k + c0 + w]

            def W2i(k, it, c0=0, w=D):
                return w2_sb[it][:, D * k + c0 : D * k + c0 + w]

            # biases: channel-major per-partition [128,1] slices
            bq_sb, bk_sb, b1_sb = [], [], []
            bqv_r = bqv.ap()
            for it in range(2):
                t = pw.tile([128, 1], F32, name=f"bq{it}", tag=f"bq{it}")
                nc.gpsimd.dma_start(
                    t[:],
                    bqv_r[0, 128 * it : 128 * it + 128].rearrange("(a b) -> a b", b=1),
                )
                bq_sb.append(t)
                t = pw.tile([128, 1], F32, name=f"bk{it}", tag=f"bk{it}")
                nc.gpsimd.dma_start(
                    t[:],
                    bqv_r[1, 128 * it : 128 * it + 128].rearrange("(a b) -> a b", b=1),
                )
                bk_sb.append(t)
            bv_bc = pw.tile([128, D], F32, name="bv_bc", tag="bv_bc")
            nc.gpsimd.dma_start(bv_bc[:], _bcast_ap(bqv.ap(), 2, D, 128))
            for it in range(8):
                t = pw.tile([128, 1], F32, name=f"b1_{it}", tag=f"b1_{it}")
                nc.gpsimd.dma_start(t[:], b1r[128 * it : 128 * it + 128, :])
                b1_sb.append(t)
            b2_bc = pw.tile([128, D], F32, name="b2_bc", tag="b2_bc")
            nc.gpsimd.dma_start(
                b2_bc[:], _bcast_ap(b2d.ap().rearrange("(a b) -> b a", b=1), 0, D, 128)
            )
            gam_bc = pw.tile([128, D], F32, name="gam_bc", tag="gam_bc")
            nc.gpsimd.dma_start(
                gam_bc[:], _bcast_ap(gam.ap().rearrange("(a b) -> b a", b=1), 0, D, 128)
            )
            eps_sb = pw.tile([128, 1], F32, name="eps_sb", tag="eps_sb")
            nc.vector.memset(eps_sb[:], LN_EPS)
            ones_sb = pw.tile([128, 1], F32, name="ones_sb", tag="ones_sb")
            nc.vector.memset(ones_sb[:], 1.0)
            ones_row = pw.tile([1, 32], F32, name="ones_row", tag="ones_row")
            nc.vector.memset(ones_row[:], 1.0)

            # residual rows, preloaded
            xr_sb = []
            for tt in range(8):
                t = pw.tile([128, D], F32, name=f"xr{tt}", tag=f"xr{tt}")
                nc.gpsimd.dma_start(t[:], xres.ap()[128 * tt : 128 * tt + 128, :])
                xr_sb.append(t)

            # ---- activation tensors ----
            q_sb, k_sb = [], []
            for it in range(2):
                t = pa.tile([128, HALF], BF16, name=f"q{it}", tag=f"q{it}")
                q_sb.append(t)
                t = pa.tile([128, L], BF16, name=f"k{it}", tag=f"k{it}")
                k_sb.append(t)
            v_aug = pa.tile([128, 16 * 264], BF16, name="v_aug", tag="v_aug")
            va = v_aug[:]
            # ones columns (col 32 of each head block per kt)
            nc.vector.tensor_copy(
                bass.AP(tensor=va.tensor, offset=va.offset + 32,
                        ap=[va.ap[0], [264, 16], [33, 8]]),
                ones_sb[:].to_broadcast((128, 16 * 8)),
            )
            # ctx_cm: [2 tiles][128, 1028]; col j <-> token s-2+j
            ctx_cm = []
            for it in range(2):
                t = pa.tile([128, 1028], BF16, name=f"ctxcm{it}", tag=f"ctxcm{it}")
                ctx_cm.append(t)
                nc.gpsimd.dma_start(t[:, 0:2], ctxh.ap()[128 * it : 128 * it + 128, 0:2])
                nc.gpsimd.dma_start(
                    t[:, 1026:1028], ctxh.ap()[128 * it : 128 * it + 128, 2:4]
                )
            y1_sb = []
            for it in range(8):
                t = pa.tile([128, 1026], BF16, name=f"y1_{it}", tag=f"y1_{it}")
                y1_sb.append(t)
            y2_sb = []
            for tt in range(8):
                t = pa.tile([128, D], F32, name=f"y2_{tt}", tag=f"y2_{tt}")
                y2_sb.append(t)

            # ---- PSUM pools (8 banks total, live for the whole kernel) ----
            psc = est.enter_context(tc.tile_pool(name="psc", bufs=2, space="PSUM"))
            pcx = est.enter_context(tc.tile_pool(name="pcx", bufs=1, space="PSUM"))
            pfil = est.enter_context(tc.tile_pool(name="pfil", bufs=3, space="PSUM"))

            # ---- cyclic SBUF pools ----
            ppr = est.enter_context(tc.tile_pool(name="ppr", bufs=10))
            pden = est.enter_context(tc.tile_pool(name="pden", bufs=2))
            pdram = est.enter_context(tc.tile_pool(name="pdram", bufs=2, space="DRAM"))
            pln = est.enter_context(tc.tile_pool(name="pln", bufs=6))
            pout = est.enter_context(tc.tile_pool(name="pout", bufs=3))

            # ================= conv emitters =================
            def emit_q(ot, ch):
                # q_sb col j <-> token s+j; x col j <-> token s-2+j
                ps = pfil.tile([128, 512], F32, name="ps_q", tag="ps_fil")
                n = 0
                for it in range(2):
                    for k in range(3):
                        nc.tensor.matmul(
                            ps[:],
                            WQi(k, it, 128 * ot, 128),
                            x_sb[it][:, 512 * ch + k + 1 : 512 * ch + k + 513],
                            start=(n == 0), stop=(n == 5),
                        )
                        n += 1
                nc.vector.tensor_scalar_add(
                    q_sb[ot][:, 512 * ch : 512 * ch + 512], ps[:], bq_sb[ot][:]
                )

            def emit_k(ot, cg):
                # k_sb col j <-> token (s-1+j) mod L
                ps = pfil.tile([128, 512], F32, name="ps_k", tag="ps_fil")
                n = 0
                for it in range(2):
                    for k in range(3):
                        nc.tensor.matmul(
                            ps[:],
                            WKi(k, it, 128 * ot, 128),
                            x_sb[it][:, 512 * cg + k : 512 * cg + k + 512],
                            start=(n == 0), stop=(n == 5),
                        )
                        n += 1
                nc.vector.tensor_scalar_add(
                    k_sb[ot][:, 512 * cg : 512 * cg + 512], ps[:], bk_sb[ot][:]
                )

            def emit_v(kt):
                # token-major, out [t(128), o(256)]
                ps = pfil.tile([128, D], F32, name="ps_v", tag="ps_fil")
                n = 0
                for it in range(2):
                    for k in range(3):
                        nc.tensor.matmul(
                            ps[:],
                            x_sb[it][:, 128 * kt + k : 128 * kt + k + 128],
                            WVi(k, it),
                            start=(n == 0), stop=(n == 5),
                        )
                        n += 1
                vv = v_aug[:]
                vout = bass.AP(
                    tensor=vv.tensor, offset=vv.offset + 264 * kt,
                    ap=[vv.ap[0], [33, 8], [1, 32]],
                )
                nc.vector.scalar_tensor_tensor(
                    vout, ps[:], 1.0, bv_bc[:], op0=ALU.mult, op1=ALU.add
                )

            def emit_conv1(ot, c0, w):
                # y1 col j <-> token s-1+j; needs ctx_cm cols [j+k]
                ps = pfil.tile([128, w], F32, name="ps_y1", tag="ps_fil")
                n = 0
                for it in range(2):
                    for k in range(3):
                        nc.tensor.matmul(
                            ps[:],
                            W1i(k, it, 128 * ot, 128),
                            ctx_cm[it][:, c0 + k : c0 + k + w],
                            start=(n == 0), stop=(n == 5),
                        )
                        n += 1
                nc.vector.tensor_scalar(
                    y1_sb[ot][:, c0 : c0 + w], ps[:],
                    b1_sb[ot][:], 0.0, op0=ALU.add, op1=ALU.max,
                )

            # conv2: per tt, 4 sub-closures sharing one PSUM tile
            c2_ps = [None] * 8

            def emit_conv2(tt, g):
                if g == 0:
                    c2_ps[tt] = pfil.tile([128, D], F32, name="ps_y2", tag="ps_fil")
                ps = c2_ps[tt]
                n = 0
                for it in range(2 * g, 2 * g + 2):
                    for k in range(3):
                        nc.tensor.matmul(
                            ps[:],
                            y1_sb[it][:, 128 * tt + k : 128 * tt + k + 128],
                            W2i(k, it),
                            start=(g == 0 and n == 0), stop=(g == 3 and n == 5),
                        )
                        n += 1
                if g == 3:
                    nc.vector.scalar_tensor_tensor(
                        y2_sb[tt][:], ps[:], 1.0, b2_bc[:],
                        op0=ALU.mult, op1=ALU.add,
                    )
                    c2_ps[tt] = None

            def emit_ln(tt):
                y2 = y2_sb[tt]
                st = pln.tile([128, 6], F32, name="st", tag="st")
                nc.vector.bn_stats(st[:], y2[:])
                mv = pln.tile([128, 2], F32, name="mv", tag="mv")
                nc.vector.bn_aggr(mv[:], st[:])
                # rsqrt(var+eps) entirely on the DVE (bit-trick seed + 2
                # Newton steps) so the Scalar engine's exp table is never
                # switched out mid-chain. Max rel err ~5e-6.
                I32 = mybir.dt.int32
                xv = pln.tile([128, 1], F32, name="xv", tag="xv")
                nc.vector.tensor_scalar_add(xv[:], mv[:, 1:2], LN_EPS)
                h = pln.tile([128, 1], I32, name="h", tag="h")
                nc.vector.tensor_scalar(
                    h[:], xv[:].bitcast(I32), 1, None,
                    op0=ALU.logical_shift_right,
                )
                hn = pln.tile([128, 1], I32, name="hn", tag="hn")
                nc.vector.tensor_scalar(
                    hn[:], h[:], -1, None, op0=ALU.bitwise_xor
                )
                rs = pln.tile([128, 1], F32, name="rs", tag="rs")
                nc.vector.tensor_scalar(
                    rs[:].bitcast(I32), hn[:], 0x5F3759E0, None, op0=ALU.add
                )
                for it_n in range(2):
                    t1 = pln.tile([128, 1], F32, name=f"t1{it_n}", tag="t1")
                    nc.vector.tensor_mul(t1[:], rs[:], rs[:])
                    t2 = pln.tile([128, 1], F32, name=f"t2{it_n}", tag="t2")
                    nc.vector.tensor_mul(t2[:], t1[:], xv[:])
                    t3 = pln.tile([128, 1], F32, name=f"t3{it_n}", tag="t3")
                    nc.vector.tensor_scalar(
                        t3[:], t2[:], -0.5, 1.5, op0=ALU.mult, op1=ALU.add
                    )
                    rn = pln.tile([128, 1], F32, name=f"rn{it_n}", tag="rn")
                    nc.vector.tensor_mul(rn[:], rs[:], t3[:])
                    rs = rn
                yn = pout.tile([128, D], F32, name="yn", tag="yn")
                nc.vector.scalar_tensor_tensor(
                    yn[:], y2[:], mv[:, 0:1], rs[:].to_broadcast((128, D)),
                    op0=ALU.subtract, op1=ALU.mult,
                )
                yg = pout.tile([128, D], F32, name="yg", tag="yg")
                nc.vector.tensor_mul(yg[:], yn[:], gam_bc[:])
                yo = pout.tile([128, D], F32, name="yo", tag="yo")
                nc.vector.tensor_add(yo[:], yg[:], xr_sb[tt][:])
                nc.gpsimd.dma_start(out.ap()[128 * tt : 128 * tt + 128, :], yo[:])

            # ================= conv emission =================
            # Emission order defines data-dependency edges only; the Tile
            # scheduler orders execution by priority. All QKV convs are
            # emitted up front at natural (low) priority; the attention
            # chain below runs in a high-priority band, so the scheduler
            # packs these conv matmuls into the PE idle under the exp chain.
            # ultra-low-priority warm-keeper matmuls: the HAM clock gate
            # halves the PE clock after any ~3.4us idle window and only
            # re-warms after a fully-busy window; these dummies feed the PE
            # in any idle it would otherwise have (they yield to ALL real
            # work via priority) so the attention-era PE never goes cold.
            def emit_dummies(n):
                with tc.high_priority(offset=-PRIO_OFF):
                    for _ in range(n):
                        dmy = pfil.tile([128, 256], F32, name="dmy", tag="ps_fil")
                        nc.tensor.matmul(
                            dmy[:], x_sb[0][:, 0:128], x_sb[1][:, 0:256],
                            start=True, stop=True,
                        )

            # Order interleaves the k0 chunks / v chunks / heads-4-7 convs
            # so each lands comfortably before its first consumer.
            emit_q(0, 0)
            emit_k(0, 0)
            emit_k(0, 1)
            emit_v(0)
            emit_v(1)
            emit_v(2)
            emit_k(0, 2)
            emit_v(3)
            emit_v(4)
            emit_k(0, 3)
            emit_v(5)
            emit_v(6)
            emit_v(7)
            for kt in range(8, 16):
                emit_v(kt)
            # heads-4-7 convs + ch1 q: not needed before ~56us sim-time;
            # metered into 28-50us so they don't displace the urgent v/k
            # fillers and the supply lasts deeper into ch0
            late = [lambda: emit_q(1, 0), lambda: emit_q(1, 1),
                    lambda: emit_k(1, 0), lambda: emit_k(1, 1),
                    lambda: emit_k(1, 2), lambda: emit_k(1, 3),
                    lambda: emit_q(0, 1)]
            for i, fn in enumerate(late):
                with tc.tile_wait_until(0.028 + 0.003 * i):
                    fn()
            # late-ch0 (~58-95us sim) has no legal conv work; metered dummy
            # matmuls keep the PE from idling into a HAM re-throttle
            for i in range(16):
                with tc.tile_wait_until(0.048 + 0.0028 * i):
                    emit_dummies(8)

            # ================= attention =================
            def normalize(cxt, ch, h0, h1):
                # denominators -> reciprocal -> DRAM roundtrip bcast
                dp = pden.tile([64, 512], F32, name="dp", tag="dp")
                nc.vector.memset(dp[:], 1.0)
                nc.vector.tensor_copy(dp[0:1, :], cxt[32:33, :])
                nc.vector.tensor_copy(dp[32:33, :], cxt[96:97, :])
                rc = pden.tile([64, 512], F32, name="rc", tag="rc")
                nc.vector.reciprocal_approx_fast(rc[:], dp[:])
                dr = pdram.tile([2, 512], F32, name="dr", tag="dr")
                rca = rc[0:1, :]
                nc.gpsimd.dma_start(
                    dr[:],
                    bass.AP(tensor=rca.tensor, offset=rca.offset,
                            ap=[[32 * rca.ap[0][0], 2]] + rca.ap[1:]),
                )
                rb = pden.tile([64, 512], F32, name="rb", tag="rb")
                for j in range(2):
                    da = dr[j : j + 1, :]
                    nc.gpsimd.dma_start(
                        rb[32 * j : 32 * j + 32, :],
                        bass.AP(tensor=da.tensor, offset=da.offset,
                                ap=[[0, 32]] + da.ap[1:]),
                    )
                for j, (p, hh) in enumerate(((0, h0), (64, h1))):
                    nc.vector.tensor_mul(
                        ctx_cm[hh // 4][
                            32 * (hh % 4) : 32 * (hh % 4) + 32,
                            2 + 512 * ch : 2 + 512 * ch + 512,
                        ],
                        cxt[p : p + 32, :],
                        rb[32 * j : 32 * j + 32, :],
                    )

            for ch in range(2):
                for hp in range(4):
                    h0, h1 = 2 * hp, 2 * hp + 1
                    ki = h0 // 4
                    p0, p1 = 32 * (h0 % 4), 32 * (h1 % 4)
                    with tc.high_priority(offset=PRIO_OFF):
                        cxt = pcx.tile([128, 512], F32, name=f"cxt{ch}{hp}",
                                       tag="cxt")

                        def spair(kt):
                            sc = psc.tile([128, 1024], F32, name="sc", tag="sc")
                            nc.tensor.matmul(
                                sc[:, 0:512],
                                k_sb[ki][p0 : p0 + 32, 128 * kt : 128 * kt + 128],
                                q_sb[ki][p0 : p0 + 32, 512 * ch : 512 * ch + 512],
                                start=True, stop=True, tile_position=(p0, 0),
                            )
                            nc.tensor.matmul(
                                sc[:, 512:1024],
                                k_sb[ki][p1 : p1 + 32, 128 * kt : 128 * kt + 128],
                                q_sb[ki][p1 : p1 + 32, 512 * ch : 512 * ch + 512],
                                start=True, stop=True, tile_position=(p1, 0),
                            )
                            return sc

                        def cpair(pr_, kt_):
                            nc.tensor.matmul(
                                cxt[0:33, :],
                                v_aug[:, 264 * kt_ + 33 * h0
                                      : 264 * kt_ + 33 * h0 + 33],
                                pr_[:, 0:512],
                                start=(kt_ == 0), stop=(kt_ == 15),
                            )
                            nc.tensor.matmul(
                                cxt[64:97, :],
                                v_aug[:, 264 * kt_ + 33 * h1
                                      : 264 * kt_ + 33 * h1 + 33],
                                pr_[:, 512:1024],
                                start=(kt_ == 0), stop=(kt_ == 15),
                            )

                        # software-pipelined: S(kt+1) is emitted right after
                        # E(kt) so the score pair issues (concurrently) during
                        # the exp and never gates the ACT chain; ctx trails.
                        sc = spair(0)
                        prev = None
                        for kt in range(16):
                            pr = ppr.tile([128, 1024], BF16, name="pr", tag="pr")
                            nc.scalar.activation(
                                pr[:], sc[:], AF.Exp, bias=0.0,
                                scale=float(INV_SQRT_H),
                            )
                            if kt < 15:
                                sc = spair(kt + 1)
                            if prev is not None:
                                cpair(*prev)
                            prev = (pr, kt)
                        cpair(*prev)
                        normalize(cxt, ch, h0, h1)
                if ch == 0:
                    # ch0 ctx complete -> FFN for y1 cols 0..511 + y2 tt 0..2
                    # metered across the ch1 window so supply doesn't drain
                    # early and starve late-ch1 into a HAM throttle
                    blk = 0
                    for r in range(2):
                        for ot in range(8):
                            with tc.tile_wait_until(0.096 + 0.0032 * blk):
                                emit_conv1(ot, 256 * r, 256)
                            blk += 1
                    for tt in range(3):
                        for g in range(4):
                            with tc.tile_wait_until(0.096 + 0.0032 * blk):
                                emit_conv2(tt, g)
                            blk += 1
                        emit_ln(tt)

            # ================= tail =================
            # remaining conv1 (cols 512..1025) + conv2 + LN
            for r, w in ((2, 256), (3, 258)):
                for ot in range(8):
                    emit_conv1(ot, 256 * r, w)
            for tt in range(3, 8):
                for g in range(4):
                    emit_conv2(tt, g)
                emit_ln(tt)

    nc.compile()
    return nc


def _host_attn_tokens(xb, toks, Wq, bq, Wk, bk, Wv, bv):
    """Attention output (pre-FFN context) rows for the given tokens, numpy."""
    k_full = np.zeros((L, D), np.float32)
    v_full = np.zeros((L, D), np.float32)
    for k in range(3):
        xs = np.roll(xb, 1 - k, axis=0)  # xs[t] = xb[(t + k - 1) % L]
        k_full += xs @ Wk[:, :, k].T
        v_full += xs @ Wv[:, :, k].T
    k_full += bk
    v_full += bv
    q8 = np.zeros((len(toks), D), np.float32)
    for k in range(3):
        idx = (toks + k - 1) % L
        q8 += xb[idx] @ Wq[:, :, k].T
    q8 += bq

    ctx8 = np.zeros((len(toks), D), np.float32)
    for h in range(H):
        sl = slice(32 * h, 32 * h + 32)
        s = (q8[:, sl] @ k_full[:, sl].T) * INV_SQRT_H  # [len, L]
        s = s - s.max(axis=1, keepdims=True)
        e = np.exp(s)
        p = e / e.sum(axis=1, keepdims=True)
        ctx8[:, sl] = p @ v_full[:, sl]
    return ctx8


def kernel(x, Wq, bq, Wk, bk, Wv, bv, W1, b1, W2, b2, gamma, beta):
    x = np.asarray(x, np.float32)
    Wq, Wk, Wv = (np.asarray(a, np.float32) for a in (Wq, Wk, Wv))
    W1, W2 = np.asarray(W1, np.float32), np.asarray(W2, np.float32)
    bq, bk, bv = (np.asarray(a, np.float32) for a in (bq, bk, bv))
    b1, b2 = np.asarray(b1, np.float32), np.asarray(b2, np.float32)
    gamma, beta = np.asarray(gamma, np.float32), np.asarray(beta, np.float32)

    if "nc" not in _cache:
        _cache["nc"] = build_nc()
    nc = _cache["nc"]

    # host-side weight transposes: [k][i][o], bf16
    wq_t = np.ascontiguousarray(Wq.transpose(2, 1, 0)).astype(NPBF16)
    wk_t = np.ascontiguousarray(Wk.transpose(2, 1, 0)).astype(NPBF16)
    wv_t = np.ascontiguousarray(Wv.transpose(2, 1, 0)).astype(NPBF16)
    w1_t = np.ascontiguousarray(W1.transpose(2, 1, 0)).astype(NPBF16)
    w2_t = np.ascontiguousarray(W2.transpose(2, 1, 0)).astype(NPBF16)
    bqv = np.stack([bq, bk, bv])

    # halo ctx (host, fp32): per batch, the 8 boundary tokens both halves need
    all_toks = np.array([2046, 2047, 1024, 1025, 1022, 1023, 0, 1])
    ctx8_by_b = [
        _host_attn_tokens(x[b], all_toks, Wq, bq, Wk, bk, Wv, bv)
        for b in range(B)
    ]
    in_maps = []
    for c in range(NCORES):
        b, half = c // 2, c % 2
        s = half * HALF
        xb = x[b]
        sel = [0, 1, 2, 3] if half == 0 else [4, 5, 6, 7]
        ctx4 = ctx8_by_b[b][sel]  # rows: s-2, s-1, e+1, e+2
        ctx4_cm = np.ascontiguousarray(ctx4.T).astype(NPBF16)  # [256, 4]

        xbT = np.ascontiguousarray(xb.T)  # [256, 2048]
        # xcm col j <-> token (s - 2 + j) mod L, j in [0, 2052)
        idx = (np.arange(L + 4) + s - 2) % L
        xcm = np.ascontiguousarray(xbT[:, idx]).astype(NPBF16)
        xres = xb[s : s + HALF] + beta[None, :]

        in_maps.append({
            "xcm": xcm,
            "xres": np.ascontiguousarray(xres),
            "wq": wq_t, "wk": wk_t, "wv": wv_t, "w1": w1_t, "w2": w2_t,
            "bqv": bqv, "b1d": b1, "b2d": b2, "gam": gamma,
            "ctxh": ctx4_cm,
        })

    res = bass_utils.run_bass_kernel_spmd(nc, in_maps, core_ids=list(range(NCORES)))
    y = np.empty((B, L, D), np.float32)
    for c in range(NCORES):
        b, half = c // 2, c % 2
        y[b, half * HALF : (half + 1) * HALF] = res.results[c]["out"]
    return y

